# revision 10
# baseline (speedup 1.0000x reference)
"""Depthwise causal Conv1d (B=4, S=4096, D=2048, K=4) on 8 TRN2 NeuronCores.

Sharding: channel-parallel — core i owns channels [i*256, (i+1)*256) for
all 4 batches over the full sequence (depthwise conv mixes nothing across
channels, so communication is zero). Each core gets 8 slabs (batch x
128-channel block) of bf16 [128, 4 + 4096] — 1.05 MB DMAs, above the
~860 KB knee where TRN2 SDMA reaches line rate — and, because every slab
spans the whole sequence, the 4 history columns are always zero (no
cross-shard halo at all). With xt[t] = x[t - 4]:
    out[p] = sum_k w_k * xt[p + k + 1] + bias.

The 4-tap accumulation runs on the TensorEngine (not hit by the TRN2
SBUF-src 2.3x errata) as diagonal matmuls accumulating in PSUM: stationary
diag(w[block, k]) [128x128] bf16, moving = a column window of the x tile.
bf16 moving operands stream 2 cols/cycle but require 4-byte-aligned (even
element) start offsets; since PSUM is fp32 (4-byte elements), odd shifts
are absorbed by the PSUM destination AP instead of the moving AP. With
ODD chunk bases C, even taps use moving start C+k+1 (even) writing
pt[:, 0:512] and odd taps use moving start C+k+2 (even) writing
pt[:, 1:512], so pt[m] consistently accumulates out[C+m]:

  chunks C in {-1, 509, ..., 3569} (one 512-f32 PSUM bank each) plus a
  17-col tail at C=4079; combine pt[1:1+N]+bias -> out[C+1 .. C+N] split
  across ACT activation (6 chunks) and DVE tensor_scalar (3 chunks), both
  PSUM-src with bf16 out. Chunk C=-1 reaches out[0], so no scalar edge
  fix-ups are needed anywhere.

Inputs ride the SP HWDGE ring, outputs + weights the ACT ring. bf16 I/O
halves HBM traffic; at ~47 us/pass the kernel moves 16.8 MB/core/pass =
~357 GB/s, at the 358 GB/s per-NeuronCore HBM limit. Products accumulate
in fp32 PSUM; measured rel err ~5e-3 vs the fp32 reference (gate 2e-2).
"""

import numpy as np

import concourse.bacc as bacc
import concourse.mybir as mybir
from concourse.bass_utils import run_bass_kernel_spmd
from concourse.tile import TileContext

B, S, D, K = 4, 4096, 2048, 4
NCORES = 8
DCORE = D // NCORES          # 256 channels per core
NSLAB = B * (DCORE // 128)   # 8 slabs of 128 channels
HIST = 4                     # history columns (K-1 needed + 1 alignment pad)
WPAD = 4128                  # slab width: 4100 used cols padded so each DRAM
                             # row is 8256 B = 129*64 B (64B-burst-aligned;
                             # the unpadded 8200 B rows cost ~15% HBM BW)
F32 = mybir.dt.float32
BF16 = mybir.dt.bfloat16
# (odd chunk base C, even-tap matmul width); odd taps/combine use width-1
CHUNKS = [(c, 512) for c in range(-1, 3570, 510)] + [(4079, 17)]
DVE_CHUNKS = (2, 5, 8)

_CACHE = {}


def _emit_pass(nc, pools, aps):
    x_d, o_d, wsb, wdg = aps
    xpool, ppool, opool = pools

    def diag(k, j):
        c = k * 2 + j
        return wdg[:, c * 128 : (c + 1) * 128]

    def bias(j):
        return wsb[:, K * 2 + j : K * 2 + j + 1]

    for s in range(NSLAB):
        j = s % 2
        xt = xpool.tile([128, WPAD], BF16, tag="xt")
        nc.sync.dma_start(out=xt[:], in_=x_d[s * 128 : (s + 1) * 128, :])
        ot = opool.tile([128, S], BF16, tag="ot")
        for ci, (C, NE) in enumerate(CHUNKS):
            NO = NE - 1
            pt = ppool.tile([128, 512], F32, tag="pt")
            for jj, k in enumerate((0, 2, 1, 3)):
                if k % 2 == 0:
                    nc.tensor.matmul(
                        pt[:, 0:NE],
                        diag(k, j),
                        xt[:, C + k + 1 : C + k + 1 + NE],
                        start=(jj == 0),
                        stop=(jj == K - 1),
                    )
                else:
                    nc.tensor.matmul(
                        pt[:, 1 : 1 + NO],
                        diag(k, j),
                        xt[:, C + k + 2 : C + k + 2 + NO],
                        start=False,
                        stop=(jj == K - 1),
                    )
            osl = ot[:, C + 1 : C + 1 + NO]
            psl = pt[:, 1 : 1 + NO]
            # ACT's PSUM-src activation is cheaper per element than DVE's
            # PSUM-src tensor_scalar, so ACT takes 6 of the 9 chunks
            if ci in DVE_CHUNKS:
                nc.vector.tensor_scalar_add(osl, psl, bias(j))
            else:
                nc.scalar.add(osl, psl, bias(j))
        nc.scalar.dma_start(out=o_d[s * 128 : (s + 1) * 128, :], in_=ot[:])


def _build_program(nreps=1):
    """nreps passes of the kernel body, fully unrolled (nreps > 1 is used
    only by test.py for steady-state timing)."""
    if nreps in _CACHE:
        return _CACHE[nreps]
    nc = bacc.Bacc("TRN2", num_devices=NCORES)
    x_d = nc.dram_tensor(
        "xin", [NSLAB * 128, WPAD], BF16, kind="ExternalInput"
    ).ap()
    w_d = nc.dram_tensor("wtab", [128, (K + 1) * 2], F32, kind="ExternalInput").ap()
    wd_d = nc.dram_tensor("wdiag", [128, K * 2 * 128], BF16, kind="ExternalInput").ap()
    o_d = nc.dram_tensor("out", [NSLAB * 128, S], BF16, kind="ExternalOutput").ap()

    with TileContext(nc) as tc:
        with (
            tc.tile_pool(name="const", bufs=1) as const,
            tc.tile_pool(name="xpool", bufs=7) as xpool,
            tc.psum_pool(name="ppool", bufs=8) as ppool,
            tc.tile_pool(name="opool", bufs=4) as opool,
        ):
            wsb = const.tile([128, (K + 1) * 2], F32, tag="wsb")
            nc.scalar.dma_start(out=wsb[:], in_=w_d)
            wdg = const.tile([128, K * 2 * 128], BF16, tag="wdg")
            nc.scalar.dma_start(out=wdg[:], in_=wd_d)

            for _ in range(nreps):
                _emit_pass(nc, (xpool, ppool, opool), (x_d, o_d, wsb, wdg))

    nc.compile()
    _CACHE[nreps] = nc
    return nc


def _shard_inputs(x, weight, bias):
    import ml_dtypes

    bf16 = ml_dtypes.bfloat16
    x = np.asarray(x, dtype=np.float32)
    weight = np.asarray(weight, dtype=np.float32)
    bias = np.asarray(bias, dtype=np.float32)

    in_maps = []
    pidx = np.arange(128)
    for core in range(NCORES):
        c0 = core * DCORE
        wr = weight[c0 : c0 + DCORE, 0, :].reshape(2, 128, K)  # [j, p, k]
        # wtab[p, k*2+j] = w[c0+j*128+p, k]; wtab[p, K*2+j] = bias
        wtab = np.empty((128, (K + 1) * 2), dtype=np.float32)
        wtab[:, : K * 2] = wr.transpose(1, 2, 0).reshape(128, K * 2)
        wtab[:, K * 2 :] = bias[c0 : c0 + DCORE].reshape(2, 128).T
        # wdiag[p, (k*2+j)*128 + q] = w[c0+j*128+p, k] * (q == p)
        wd = np.zeros((128, K * 2, 128), dtype=np.float32)
        for k in range(K):
            for j in range(2):
                wd[pidx, k * 2 + j, pidx] = wr[j, :, k]
        wdiag = wd.reshape(128, K * 2 * 128).astype(bf16)

        # slab s = b*2 + j; full sequences, so history cols are always
        # zero; cols beyond 4+S are 64B-alignment padding, never read
        xc = np.zeros((NSLAB * 128, WPAD), dtype=bf16)
        for b in range(B):
            xbt = x[b].T  # [D, S] view
            for j in range(2):
                s = b * 2 + j
                xc[s * 128 : (s + 1) * 128, HIST : HIST + S] = xbt[
                    c0 + j * 128 : c0 + (j + 1) * 128, :
                ]
        in_maps.append({"xin": xc, "wtab": wtab, "wdiag": wdiag})
    return in_maps


def _run(x, weight, bias, trace=False):
    nc = _build_program()
    in_maps = _shard_inputs(x, weight, bias)
    res = run_bass_kernel_spmd(nc, in_maps, list(range(NCORES)), trace=trace)
    out = np.empty((B, S, D), dtype=np.float32)
    for core in range(NCORES):
        c0 = core * DCORE
        r = res.results[core]["out"].astype(np.float32)  # [1024, S]
        for b in range(B):
            for j in range(2):
                s = b * 2 + j
                out[b, :, c0 + j * 128 : c0 + (j + 1) * 128] = r[
                    s * 128 : (s + 1) * 128, :
                ].T
    return out, res


def kernel(x, weight, bias):
    out, _ = _run(x, weight, bias, trace=False)
    return out
